# revision 40
# baseline (speedup 1.0000x reference)
"""Distributed Trainium2 Bass kernel for AdaptedAttention (LLaMA-Adapter style).

Sharding: pure data-parallel over the B*S = 8192 token axis (1024 tokens per
core across 8 NeuronCores).  The adapter attention only attends to the L=64
adapter slots, so there is no cross-token dependency; each core produces its
own slice of the output with no collectives.

Algebraic restructure vs the straightforward formulation:
  - Wo is folded into the adapter values on host: VWo_h = V_h @ Wo_h
    ([L=64, HID] per head).  Since L < D, the output-side GEMM
    out = sum_h probs_h @ VWo_h costs half the MACs of (probs @ V) @ Wo and
    eliminates the aoT intermediate entirely.
  - Adapter K / V / VWo are prompt-side (length-L, token-independent)
    precompute on host, like the RoPE tables; base_output is added on host.
  - RoPE rotate-half is eliminated: scores contract over the head dim, so
    scores = KT^T (q*cos) + KTrowswap^T (q*sin'); both arms are the two
    K-groups of one fp8 DoubleRow matmul.

Precision: all GEMMs fp8e4 DoubleRow (fp32 PSUM, one output column/cycle =
2x fp8 rate); softmax in bf16/f32; total rel err ~7e-3 vs the 2e-2 gate.

Softmax denominators are batched: each head-PAIR's exps live in opposite
halves of a [128, 512] tile, and one ones-like matmul per (pair, m)
accumulates both heads' sums into rows of a per-4-head-group [8, 512] PSUM
tile (indicator weights scaled 1/32 = the probs fp8 scale), so one
reciprocal serves 8 rows.  Reciprocals are copied to DRAM, and per-head
1/sum broadcasts run as stride-0-source DMAs (any partition target,
parallel across rings); DVE multiplies in bf16 and the scalar engine casts
to fp8 (the DVE fp8-out path is ~3x slower).

Device pipeline per core (single fused graph):
  - per head h: 16 fp8 DR matmuls (K=256) accumulate qT from 16 streamed
    xT k-chunks; DVE applies cos/sin (scales folded into host tables) writing
    fp8 qa|qb straight to SBUF (no DRAM roundtrip); scores lag 3 heads,
    denominator sums one further slot (so exp never stalls TensorE).
  - per 4-head group: reciprocal -> DRAM -> DMA broadcasts -> probs into
    per-head-group fp8 tiles (DoubleRow rhs layout, 4 heads per K=256).
  - output GEMM: outT[nblk] = sum_hg VWo_hg^T probs_hg via 512 DR matmuls
    in software-pipelined [2,1]-nblk groups -- each group's last (hg7)
    accumulation is deferred past the next group's hg0-6 matmuls, giving a
    standing ~40-matmul pre-issue window that rides out the tail softmax
    latency within 6 PSUM banks (the other 2 hold in-flight denominators).
Host: adds base_output and descales (exact f32).
"""

import numpy as np
import ml_dtypes

B, S, HID = 4, 2048, 4096
H, D, L = 32, 128, 64
NCORES = 8
T = B * S
TC = T // NCORES          # tokens per core (1024)
KC = HID // 128           # 32 contraction chunks over hidden dim
MB = 2                    # 512-token m-chunks per core
ROPE_THETA = 10000.0

S_X = 16.0                # fp8 scale on xT
S_Q = 8192.0              # fp8 scale on WqT (1/sqrt(D) already folded)
S_P = 16.0                # fp8 scale on qa/qb (rope products)
S_K = 16.0                # fp8 scale on adapter KT
QSCALE = S_P / (S_X * S_Q)   # folded into the cos/sin tables on host
ESCALE = 1.0 / (S_P * S_K)   # descale via the exp activation's scale arg
S_PRB = 32.0              # probs fp8 scale (folded into the sums weights)
S_VW = 2048.0             # fp8 scale on VWo (validated against max on host)

GH = 4                    # heads per softmax-denominator group
NR = 2 * GH               # denominator rows per group (j%GH, m)
NG = H // GH              # number of groups (8)
LAG = 3                   # attention stages run LAG heads behind Q-proj

_cache = {}


def _build(tc_tokens=TC):
    """Builds the SPMD Bass graph (identical on all 8 cores)."""
    import concourse.tile as tile
    from concourse import bacc, mybir
    from contextlib import ExitStack

    bf16 = mybir.dt.bfloat16
    fp8 = mybir.dt.float8e4
    f32 = mybir.dt.float32
    MUL = mybir.AluOpType.mult
    EXP = mybir.ActivationFunctionType.Exp
    DR = mybir.MatmulPerfMode.DoubleRow

    assert tc_tokens == MB * 512

    nc = bacc.Bacc(
        "TRN2",
        target_bir_lowering=False,
        debug=False,
        enable_asserts=False,
        num_devices=NCORES,
    )

    # Host-pretiled layouts (every DMA a large contiguous burst):
    #   xT    16 chunks [128, 2*tc]: chunk s, [p, (m, i, 512)] DR rhs
    #   wqT   [H*128, KC*128] : [128h+p, 256k2+128i+c] = Wq.T[256k2+128i+p, 128h+c]
    #   ktp   [128, H*2*L]    : per head [KT*S_K | KTswap*S_K] K-groups
    #   vwo   [128, KC*8*2*128]: [l2, (n, hg, i, c)] fp8 DoubleRow lhsT blocks
    #   eyes  [64, 16*16] bf16: block r = indicator column r scaled 1/S_PRB
    NCH = 16                             # xT k-chunks
    XCW = (KC // NCH) * tc_tokens        # columns per xT chunk
    xTs = [nc.dram_tensor(f"xT{s}", [128, XCW], fp8, kind="ExternalInput").ap()
           for s in range(NCH)]
    wqT = nc.dram_tensor("wqT", [H * 128, KC * 128], fp8, kind="ExternalInput").ap()
    vwo = nc.dram_tensor("vwo", [128, KC * 8 * 256], fp8, kind="ExternalInput").ap()
    ktp = nc.dram_tensor("ktp", [128, H * 2 * L], fp8, kind="ExternalInput").ap()
    cosT = nc.dram_tensor("cosT", [D, tc_tokens], bf16, kind="ExternalInput").ap()
    sinT = nc.dram_tensor("sinT", [D, tc_tokens], bf16, kind="ExternalInput").ap()
    eyesT = nc.dram_tensor("eyesT", [128, 4 * NR], bf16, kind="ExternalInput").ap()
    outT = nc.dram_tensor("outT", [HID, tc_tokens], bf16, kind="ExternalOutput").ap()

    with tile.TileContext(nc) as tc, ExitStack() as ctx:
        persist = ctx.enter_context(tc.tile_pool(name="persist", bufs=1))

        # ---- persistent SBUF residents ----
        xT_sb = [persist.tile([128, XCW], fp8, name=f"xT{s}") for s in range(NCH)]
        cos_sb = persist.tile([128, tc_tokens], bf16)
        sin_sb = persist.tile([128, tc_tokens], bf16)
        ktp_sb = persist.tile([128, H * 2 * L], fp8)
        eyes_sb = persist.tile([128, 4 * NR], bf16)
        # probs, fp8 DoubleRow rhs layout, one tile per output-GEMM head
        # group so GEMM matmuls only depend on their own group's writes:
        # head j -> (hg=j//4, i=(j%4)//2, parity=j%2); col m*1024 + 512*i,
        # partitions 64*parity+.
        probs_sb = [persist.tile([128, MB * 2 * 512], fp8, name=f"pr{hg}")
                    for hg in range(8)]

        with tc.tile_pool(name="wq", bufs=3) as wqp, \
             tc.tile_pool(name="qab", bufs=12) as qabp, \
             tc.tile_pool(name="esb", bufs=24) as esbp, \
             tc.tile_pool(name="attn", bufs=4) as asb, \
             tc.tile_pool(name="qps", bufs=4, space="PSUM") as qpsp, \
             tc.tile_pool(name="scps", bufs=2, space="PSUM") as scp, \
             tc.tile_pool(name="sups", bufs=2, space="PSUM") as sup, \
             tc.tile_pool(name="recdp", bufs=2, space="DRAM") as dramp:

            NBG = 3      # output-GEMM nblk group size (6 PSUM banks)
            qab_st, esb_st, sums_st, rec_st, wq_st, vw_st = ({}, {}, {}, {},
                                                             {}, {})
            pair_st = {}

            def vw_fetch(ni):
                vw_sb = persist.tile([128, 8 * 256], fp8, tag="vw",
                                     name=f"vw{ni}", bufs=8)
                nc.sync.dma_start(vw_sb[:],
                                  vwo[:, 2048 * ni:2048 * (ni + 1)])
                vw_st[ni] = vw_sb

            def wq_fetch(h, split=False):
                wq_sb = wqp.tile([128, KC * 128], fp8, tag="wq",
                                 name=f"wq{h}")
                if split:   # first head: land the low k2 half sooner
                    hw = KC * 64
                    nc.sync.dma_start(wq_sb[:, 0:hw],
                                      wqT[128 * h:128 * (h + 1), 0:hw])
                    nc.sync.dma_start(wq_sb[:, hw:2 * hw],
                                      wqT[128 * h:128 * (h + 1), hw:2 * hw])
                wq_st[h] = wq_sb
                if not split:
                    nc.sync.dma_start(wq_sb[:],
                                      wqT[128 * h:128 * (h + 1), :])

            # sync ring: wq0, odd xT chunks, wq1, rope tables; scalar
            # ring: even xT chunks -- ordered so each k2-chunk and table
            # lands just before its first consumer
            wq_fetch(0, split=True)
            for s in range(0, NCH, 2):
                nc.scalar.dma_start(xT_sb[s][:], xTs[s][:])
            for s in (1, 3):
                nc.sync.dma_start(xT_sb[s][:], xTs[s][:])
            wq_fetch(1)        # before the later chunks: head 1 must not
                               # stall on weights behind 11us of activations
            for s in range(5, NCH, 2):
                nc.sync.dma_start(xT_sb[s][:], xTs[s][:])
            nc.sync.dma_start(cos_sb[:], cosT[:])
            nc.sync.dma_start(sin_sb[:], sinT[:])
            nc.sync.dma_start(ktp_sb[:], ktp[:])
            nc.sync.dma_start(eyes_sb[:], eyesT[:])

            def qproj(h):
                if h + 2 < H:
                    wq_fetch(h + 2)
                wq_sb = wq_st.pop(h)
                wq_r = wq_sb.rearrange("p (k i c) -> p k i c", k=KC // 2, i=2)
                qps = [qpsp.tile([128, 512], f32, tag="qp", name=f"qp{h}_{m}")
                       for m in range(MB)]
                for k2 in range(KC // 2):
                    nkc = KC // (2 * NCH)
                    s, kl = k2 // nkc, k2 % nkc
                    x_r = xT_sb[s].rearrange("p (k q i m) -> p k q i m",
                                             k=nkc, q=MB, i=2)
                    for m in range(MB):
                        nc.tensor.matmul(
                            qps[m][:], wq_r[:, k2], x_r[:, kl, m],
                            start=(k2 == 0), stop=(k2 == KC // 2 - 1),
                            perf_mode=DR,
                        )
                # RoPE products straight to fp8 SBUF (DoubleRow rhs layout
                # [qa(512) | qb(512)]); per-(h,m) tiles so each scores
                # matmul waits on only its own two DVE ops
                for m in range(MB):
                    ms = slice(512 * m, 512 * (m + 1))
                    qab = qabp.tile([128, 1024], fp8, tag="qab",
                                    name=f"qab{h}_{m}")
                    nc.vector.tensor_tensor(
                        qab[:, 0:512], qps[m][:], cos_sb[:, ms], MUL)
                    nc.vector.tensor_tensor(
                        qab[:, 512:1024], qps[m][:], sin_sb[:, ms], MUL)
                    qab_st[(h, m)] = qab

            def attn_sc(j):     # scores + exp
                kt_h = ktp_sb.rearrange("p (h i l) -> p h i l", h=H, i=2)[:, j]
                for m in range(MB):
                    qab = qab_st.pop((j, m))
                    sc = scp.tile([64, 512], f32, tag="sc", name=f"sc{j}_{m}")
                    nc.tensor.matmul(
                        sc[:], kt_h,
                        qab[:].rearrange("p (i m) -> p i m", i=2),
                        start=True, stop=True, perf_mode=DR,
                    )
                    p, half = j // 2, j % 2
                    if half == 0:
                        pair_st[(p, m)] = esbp.tile([128, 512], bf16,
                                                    tag="esb",
                                                    name=f"esb{p}_{m}")
                    esb = pair_st[(p, m)][64 * half:64 * half + 64, :]
                    nc.scalar.activation(esb, sc[:], EXP, scale=ESCALE)

            def attn_sum(p):    # grouped denominator, one matmul per
                                # head-pair (K=128 over both heads' exps)
                g, q = (2 * p) // GH, p % 2
                if q == 0:
                    sums_st[g] = sup.tile([NR, 512], f32, tag="sums",
                                          name=f"su{g}")
                for m in range(MB):
                    b = 2 * q + m
                    nc.tensor.matmul(
                        sums_st[g][:], eyes_sb[:, NR * b:NR * (b + 1)],
                        pair_st[(p, m)][:],
                        start=(b == 0), stop=(b == 3))

            def normalize(g):
                # one reciprocal per group; a DRAM copy lets per-head
                # broadcasts run as parallel DMAs (any partition target)
                rec = asb.tile([NR, 512], bf16, tag="rec", name=f"re{g}",
                               bufs=2)
                with nc.allow_low_precision(reason="bf16 softmax weights"):
                    nc.vector.reciprocal(rec[:], sums_st.pop(g)[:])
                recd = dramp.tile([NR, 512], bf16, tag="recd",
                                  name=f"rd{g}", bufs=2)
                # sync ring: a scalar-ring DMA here would block the FIFO
                # behind the reciprocal and stall the remaining exps
                nc.sync.dma_start(recd[:], rec[:])
                rec_st[g] = recd

            def probs(j, m):
                g, r = j // GH, (j % GH) * 2 + m
                recd = rec_st[g]
                p, half = j // 2, j % 2
                hs = slice(64 * half, 64 * half + 64)
                esb = pair_st[(p, m)][hs, :]
                bc = asb.tile([128, 512], bf16, tag="bc", name=f"bc{j}_{m}",
                              bufs=6)
                # same ring as the recd write: HWDGE is FIFO per SDMA
                # engine, so the read can never overtake the write landing
                nc.sync.dma_start(bc[hs, :],
                                  recd[r:r + 1, :].to_broadcast([64, 512]))
                # bf16 multiply on DVE (fp8-out DVE path is ~3x slower);
                # fp8 conversion rides the underused scalar engine
                pbf = asb.tile([128, 512], bf16, tag="pbf", name=f"pb{j}_{m}",
                               bufs=6)
                nc.vector.tensor_tensor(pbf[hs, :], esb, bc[hs, :], MUL)
                hg, i, par = j // 4, (j % 4) // 2, j % 2
                col = m * 1024 + 512 * i
                nc.scalar.copy(
                    probs_sb[hg][64 * par:64 * (par + 1), col:col + 512],
                    pbf[hs, :])

            # -------- main pipeline over heads --------
            ops = []
            for h in range(H + LAG + 2):
                if h < H:
                    ops.append(("qproj", h))
                jj = h - LAG
                if 0 <= jj < H:
                    ops.append(("attn_sc", jj))
                js = h - LAG - 1
                if 0 <= js < H and js % 2 == 1:
                    ops.append(("attn_sum", js // 2))
                    if js % GH == GH - 1:
                        ops.append(("norm", js // GH))

            pending = []        # (j, m) probs not yet emitted
            for op, a in ops:
                if op == "qproj":
                    qproj(a)
                    if a == H - 3:
                        for ni in range(2 * NBG):
                            vw_fetch(ni)
                elif op == "attn_sc":
                    attn_sc(a)
                elif op == "attn_sum":
                    attn_sum(a)
                else:
                    normalize(a)
                    if a == NG - 1:
                        # tail group: m-major order so the output GEMM's
                        # m=0 PSUM banks can close as early as possible
                        pending.extend((a * GH + t, m)
                                       for m in range(MB) for t in range(GH))
                    else:
                        pending.extend((a * GH + t, m)
                                       for t in range(GH) for m in range(MB))
                # trickle probs work between heads (2 per slot keeps the
                # gpsimd queue fed without bunching)
                if op == "qproj":
                    for _ in range(3):
                        if pending:
                            j, m = pending.pop(0)
                            probs(j, m)
            for j, m in pending:
                probs(j, m)

        # ------- output GEMM: outT[nblk] = sum_hg VWo_hg^T probs_hg -------
        # (pools shared with the main block; vw weights prefetched during
        # the main-loop epilogue so the first matmuls have data ready)
        if True:
            probs_r = [t.rearrange("p (m i c) -> p m i c", m=MB, i=2)
                       for t in probs_sb]
            # alternating [2,1] groups: adjacent pairs fit 6 PSUM banks, so
            # each group's hg7 pass can be deferred until AFTER the next
            # group's hg0-6 matmuls -- a standing ~42-matmul pre-issue window
            # that rides out the tail softmax-normalize latency
            bounds = [0]
            while bounds[-1] < KC:
                bounds.append(min(bounds[-1] + (2 if len(bounds) % 2 else 1),
                                  KC))
            if bounds[-1] - bounds[-2] == 2:
                bounds.insert(-1, bounds[-1] - 1)   # 1-nblk final group
            groups = list(zip(bounds[:-1], bounds[1:]))

            def emit_front(nb0, nbe):
                vws, opss = [], []
                for ni in range(nb0, nbe):
                    vws.append(vw_st.pop(ni))
                    if ni + 6 < KC:
                        vw_fetch(ni + 6)
                    opss.append([opp.tile([128, 512], f32, tag="op",
                                          name=f"op{ni}_{m}")
                                 for m in range(MB)])
                for g, ni in enumerate(range(nb0, nbe)):
                    vw_r = vws[g].rearrange("p (hg i c) -> p hg i c",
                                            hg=8, i=2)
                    for hg in range(7):
                        for m in range(MB):
                            nc.tensor.matmul(
                                opss[g][m][:], vw_r[:, hg], probs_r[hg][:, m],
                                start=(hg == 0), stop=False,
                                perf_mode=DR,
                            )
                return vws, opss

            def emit_close(nb0, nbe, vws, opss):
                for g, ni in enumerate(range(nb0, nbe)):
                    vw_r = vws[g].rearrange("p (hg i c) -> p hg i c",
                                            hg=8, i=2)
                    osb = fin.tile([128, MB * 512], bf16, tag="osb")
                    for m in range(MB):
                        nc.tensor.matmul(
                            opss[g][m][:], vw_r[:, 7], probs_r[7][:, m],
                            start=False, stop=True,
                            perf_mode=DR,
                        )
                    nc.scalar.copy(osb[:, 0:512], opss[g][0][:])
                    nc.sync.dma_start(
                        outT[128 * ni:128 * (ni + 1), 0:512], osb[:, 0:512])
                    with nc.allow_low_precision(reason="bf16 out"):
                        nc.vector.tensor_scalar_mul(osb[:, 512:1024],
                                                    opss[g][1][:], 1.0)
                    nc.sync.dma_start(
                        outT[128 * ni:128 * (ni + 1), 512:1024],
                        osb[:, 512:1024])

            prev = None
            for nb0, nbe in groups:
                front = emit_front(nb0, nbe)
                if prev is not None:
                    emit_close(prev[0], prev[1], *prev[2])
                prev = (nb0, nbe, front)
            emit_close(prev[0], prev[1], *prev[2])

    nc.compile()
    return nc


def _host_prep(hidden_states, base_output, Wq, Wk, Wv, Wo, adaption_prompt,
               adaption_gate, position_ids, tc_tokens=TC, ncores=NCORES):
    bf16 = ml_dtypes.bfloat16
    fp8 = ml_dtypes.float8_e4m3
    f32 = np.float32

    x = np.ascontiguousarray(np.asarray(hidden_states, f32).reshape(T, HID))
    pos = np.asarray(position_ids).reshape(T).astype(np.int64)

    inv = 1.0 / (ROPE_THETA ** (np.arange(0, D, 2, dtype=f32) / D))
    freqs = pos[:, None].astype(f32) * inv[None, :]          # [T, 64]
    emb = np.concatenate([freqs, freqs], axis=1)             # [T, 128]
    # QSCALE compensates the fp8 scaling of the Q projection inputs
    cos = (np.cos(emb) * QSCALE).astype(f32)
    sin = (np.sin(emb) * QSCALE).astype(f32)
    # sin arm pairs with the row-swapped KT: +sin (p<64), -sin (p>=64)
    sin_signed = sin.copy()
    sin_signed[:, D // 2:] *= -1.0

    gate = f32(np.asarray(adaption_gate).reshape(-1)[0])
    scale = f32(1.0 / np.sqrt(D))

    def tile_doublerow(A):
        # A [HID, HID] -> [KC*128, KC*128] with
        # [128n+p, 256k2+128i+c] = A[256k2+128i+p, 128n+c]
        return np.ascontiguousarray(
            A.reshape(KC // 2, 2, 128, KC, 128).transpose(3, 2, 0, 1, 4)
             .reshape(KC * 128, KC * 128))

    def tile_dr_rhs(A):
        # A [HID, N] -> [128, KC*N], cols (k2, mc, i, m):
        # [p, k2*2N + mc*1024 + i*512 + m] = A[256k2+128i+p, 512mc+m]
        n = A.shape[1]
        return np.ascontiguousarray(
            A.reshape(KC // 2, 2, 128, n // 512, 512)
             .transpose(2, 0, 3, 1, 4).reshape(128, KC * n))

    WqT = tile_doublerow(np.asarray(Wq, f32).T * (scale * f32(S_Q))).astype(fp8)

    # ---- prompt-side precompute (token-independent, like the RoPE tables) --
    prompt = np.asarray(adaption_prompt, f32).reshape(L, HID)
    K = (prompt @ np.asarray(Wk, f32).T).reshape(L, H, D)    # [L, H, D]
    V = (prompt @ np.asarray(Wv, f32).T).reshape(L, H, D) * gate
    # ktp: per head [KT*S_K | KTswap*S_K] as the two fp8-DoubleRow K-groups
    KT = K.transpose(2, 1, 0) * f32(S_K)                     # [D, H, L]
    KTs = np.concatenate([KT[D // 2:], KT[:D // 2]], axis=0)
    ktp = np.stack([KT, KTs], axis=2)                        # [D, H, 2, L]
    ktp = np.ascontiguousarray(ktp.transpose(0, 1, 2, 3)
                               .reshape(D, H * 2 * L)).astype(fp8)
    # VWo[h] = V_h @ Wo_h  [L, HID];  Wo_h = Wo.T[128h:128h+128, :]
    WoT = np.asarray(Wo, f32).T
    VW = np.einsum("lhd,hdn->hln", V, WoT.reshape(H, D, HID), optimize=True)
    vw_scale = f32(S_VW)
    mx = np.abs(VW).max()
    if mx * vw_scale > 224.0:
        vw_scale = f32(224.0 / mx)
    # DoubleRow lhsT blocks: head j=4hg+2i+par contributes at partitions
    # 64par+l of K-group i; lhsT[p, ni, hg, i, c] = VWo_j[l, 128ni+c]
    vwo = np.zeros((128, KC, 8, 2, 128), np.float32)
    VWg = (VW * vw_scale).reshape(8, 2, 2, L, KC, 128)   # [hg, i, par, ...]
    for hg in range(8):
        for i in range(2):
            for par in range(2):
                vwo[64 * par:64 * par + L, :, hg, i, :] = VWg[hg, i, par]
    vwo = np.ascontiguousarray(vwo.reshape(128, KC * 8 * 256)).astype(fp8)

    # eyes: per (pair-in-group q, m) block [128, NR]: rows 0-63 (head 2q)
    # hit column 4q+m, rows 64-127 (head 2q+1) hit column 4q+2+m; value
    # 1/S_PRB folds the probs fp8 scale
    NR = 2 * GH
    eyes = np.zeros((128, 4, NR), np.float32)
    for q in range(2):
        for m in range(2):
            b = 2 * q + m
            eyes[0:64, b, 4 * q + m] = 1.0 / S_PRB
            eyes[64:128, b, 4 * q + 2 + m] = 1.0 / S_PRB
    eyesT = eyes.reshape(128, 4 * NR).astype(bf16)

    NCH = 16
    in_maps = []
    for c in range(ncores):
        lo = c * tc_tokens
        hi = lo + tc_tokens
        xc = tile_dr_rhs((x[lo:hi].T * f32(S_X)).astype(fp8))
        xw = xc.shape[1] // NCH
        im = {
            "wqT": WqT,
            "vwo": vwo,
            "ktp": ktp,
            "eyesT": eyesT,
            "cosT": np.ascontiguousarray(cos[lo:hi].T).astype(bf16),
            "sinT": np.ascontiguousarray(sin_signed[lo:hi].T).astype(bf16),
        }
        for s in range(NCH):
            im[f"xT{s}"] = np.ascontiguousarray(xc[:, s * xw:(s + 1) * xw])
        in_maps.append(im)
    return in_maps, float(vw_scale)


def kernel(hidden_states, base_output, Wq, Wk, Wv, Wo, adaption_prompt,
           adaption_gate, position_ids):
    from concourse import bass_utils

    if "nc" not in _cache:
        _cache["nc"] = _build()
    nc = _cache["nc"]

    in_maps, vw_scale = _host_prep(
        hidden_states, base_output, Wq, Wk, Wv, Wo, adaption_prompt,
        adaption_gate, position_ids)

    res = bass_utils.run_bass_kernel_spmd(nc, in_maps,
                                          core_ids=list(range(NCORES)))

    base = np.asarray(base_output, np.float32).reshape(T, HID)
    oscale = np.float32(1.0 / (vw_scale * S_PRB))
    out = np.empty((T, HID), np.float32)
    for c in range(NCORES):
        sl = slice(c * TC, (c + 1) * TC)
        out[sl] = base[sl] + res.results[c]["outT"].T.astype(np.float32) * oscale
    return out.reshape(B, S, HID)
